# revision 13
# baseline (speedup 1.0000x reference)
"""Chamfer loss on 8 Trainium2 NeuronCores — windowed candidates + exact
tier-2 fallback.

Data-parallel over batch B=8: core c handles batch element c.

Host preprocessing (per batch element): sort both point sets by their
x-coordinate.  The x-gap lower-bounds the Euclidean distance, so a
query's nearest neighbour lies within x-rank window whose width scales
with its NN distance.  A cheap probe (distance to the +-128 rank
neighbours, O(N*256) host flops) yields a certified upper bound r_ub on
every point's NN distance, hence a certified candidate rank interval
[x - r_ub, x + r_ub].  Points whose interval fits their block's fixed
512-wide tier-1 window (>99% of points) are exactly solved by tier 1;
the few dozen others are exactly solved by a padded 128-query tier-2
full scan per direction.  The final result is exact up to fp16 rounding
of individual distances.

Device algorithm (per core):
  tier 1: for each of 64 query blocks (128 sorted a-points), ONE K=24
  bf16 matmul (fp32 coords split into 3 bf16 components each: 6
  cross-product rows per coordinate + 3 rows per squared norm keep
  ~1e-7 absolute accuracy at full PE rate) produces the [128, 512]
  fp32 distance tile in PSUM.  ScalarE converts it to fp16 in SBUF.
  VectorE row-min-reduces it with one tensor_tensor_scan (min,min,
  stride-0 broadcast output) -> a->b minima, and folds it into a
  persistent per-lane column-min accumulator with one 2x-mode
  tensor_tensor min -> b->a partial minima.  Each distance value costs
  one ScalarE touch + one DVE cycle.  The accumulator [128, 8192] is
  finished with PE transposes (fp16 -> fp16 PSUM) + strided
  tensor_reduce mins.
  tier 2: 128 gathered queries per direction scan all 8192 candidates
  (16 matmuls, chained scans).
Per-point minima ship to the host, which substitutes tier-2 values for
the flagged points and does relu/sqrt/mean in fp64.
"""

import numpy as np
import ml_dtypes

import concourse.bass as bass
import concourse.mybir as mybir
import concourse.tile as tile
from concourse import bacc
from concourse.bass_utils import run_bass_kernel_spmd

B = 8
N = 8192          # points per set
K = 24            # augmented contraction rows
NT = N // 128     # 64 blocks of 128 sorted points
W1 = 512          # tier-1 candidate window per query block
T2 = 128          # tier-2 queries per direction (padded)
T2C = 512         # tier-2 candidate chunk (PSUM-bank-limited matmul width)
TG = 8            # accumulator tiles per transpose group
KPROBE = 128      # host probe: +-KPROBE rank neighbours bound the NN dist
F32 = mybir.dt.float32
F16 = mybir.dt.float16
BF16 = mybir.dt.bfloat16
BF = ml_dtypes.bfloat16
BIG = 60000.0     # fp16-safe "infinity"

_NC_CACHE = None


def _split3(v32: np.ndarray):
    """fp32 -> (hi, mid, lo) bf16 triple with hi+mid+lo == v to ~2^-24 rel."""
    v1 = v32.astype(BF)
    r = v32 - v1.astype(np.float32)
    v2 = r.astype(BF)
    v3 = (r - v2.astype(np.float32)).astype(BF)
    return v1, v2, v3


def _w_side(pts: np.ndarray):
    """pts [n,3] fp32 -> w [24,n] bf16 stationary-side operand.

    Row pairing (per coordinate k, g = split3(-2*coord), h = split3(coord)):
      w rows: g1 g1 g2 g2 g1 g3   (m rows: h1 h2 h1 h2 h3 h1)
    so sum_r w[r]*m[r] = -2*coord_a*coord_b up to ~2^-26 terms.
    Rows 18-20: split3(||.||^2) against ones; rows 21-23: ones against the
    other side's split3(||.||^2).
    """
    s = (pts.astype(np.float64) ** 2).sum(axis=1).astype(np.float32)
    s1, s2, s3 = _split3(s)
    w = np.empty((K, pts.shape[0]), dtype=BF)
    for k in range(3):
        c = pts[:, k].astype(np.float32)
        g1, g2, g3 = _split3(-2.0 * c)
        r = 6 * k
        w[r + 0], w[r + 1], w[r + 2] = g1, g1, g2
        w[r + 3], w[r + 4], w[r + 5] = g2, g1, g3
    one = np.ones(pts.shape[0], dtype=BF)
    w[18], w[19], w[20] = s1, s2, s3
    w[21], w[22], w[23] = one, one, one
    return w


def _m_side(pts: np.ndarray):
    """pts [n,3] fp32 -> m [24,n] bf16 moving-side operand (see _w_side)."""
    s = (pts.astype(np.float64) ** 2).sum(axis=1).astype(np.float32)
    s1, s2, s3 = _split3(s)
    m = np.empty((K, pts.shape[0]), dtype=BF)
    for k in range(3):
        c = pts[:, k].astype(np.float32)
        h1, h2, h3 = _split3(c)
        r = 6 * k
        m[r + 0], m[r + 1], m[r + 2] = h1, h2, h1
        m[r + 3], m[r + 4], m[r + 5] = h2, h3, h1
    one = np.ones(pts.shape[0], dtype=BF)
    m[18], m[19], m[20] = one, one, one
    m[21], m[22], m[23] = s1, s2, s3
    return m


def _win_start(i: int) -> int:
    """Tier-1 window start (rank-centred on block i, clamped)."""
    return min(max(i * 128 + 64 - W1 // 2, 0), N - W1)


def _build_nc():
    nc = bacc.Bacc(None)
    wa_d = nc.declare_dram_parameter("wa", [K, N], BF16, isOutput=False)
    mb_d = nc.declare_dram_parameter("mb", [K, N], BF16, isOutput=False)
    ma_d = nc.declare_dram_parameter("ma", [K, N], BF16, isOutput=False)
    w2a_d = nc.declare_dram_parameter("w2a", [K, T2], BF16, isOutput=False)
    w2b_d = nc.declare_dram_parameter("w2b", [K, T2], BF16, isOutput=False)
    eye_d = nc.declare_dram_parameter("eye", [128, 128], F32, isOutput=False)
    sa_d = nc.declare_dram_parameter("sa", [128, NT], F32, isOutput=True)
    sb_d = nc.declare_dram_parameter("sb", [128, NT], F32, isOutput=True)
    t2_d = nc.declare_dram_parameter("t2", [2, 128], F32, isOutput=True)

    MIN = mybir.AluOpType.min
    NG = NT // TG  # transpose groups

    with tile.TileContext(nc) as tc:
        with (
            tc.tile_pool(name="const", bufs=1) as cpool,
            tc.tile_pool(name="psum", bufs=3, space="PSUM") as pspool,
            tc.tile_pool(name="psum2", bufs=2, space="PSUM") as ps2pool,
            tc.tile_pool(name="tpsum", bufs=1, space="PSUM") as tppool,
            tc.tile_pool(name="scopy", bufs=4) as sbpool,
            tc.tile_pool(name="scopy2", bufs=2) as sb2pool,
        ):
            wa_t = cpool.tile([K, N], BF16, tag="wa")
            mb_t = cpool.tile([K, N], BF16, tag="mb")
            ma_t = cpool.tile([K, N], BF16, tag="ma")
            w2a_t = cpool.tile([K, T2], BF16, tag="w2a")
            w2b_t = cpool.tile([K, T2], BF16, tag="w2b")
            eyef_t = cpool.tile([128, 128], F32, tag="eyef")
            eye_t = cpool.tile([128, 128], F16, tag="eye")
            # chunked so the first tier-1 unit starts after ~1/4 of the load
            for t, dram in ((wa_t, wa_d), (mb_t, mb_d)):
                for h in range(4):
                    nc.sync.dma_start(out=t[:, h * N // 4:(h + 1) * N // 4],
                                      in_=dram[:, h * N // 4:(h + 1) * N // 4])
            for t, dram in ((ma_t, ma_d), (w2a_t, w2a_d), (w2b_t, w2b_d),
                            (eyef_t, eye_d)):
                nc.sync.dma_start(out=t[:], in_=dram[:])
            nc.scalar.copy(out=eye_t[:], in_=eyef_t[:])

            # persistent per-lane column-min accumulator (b->a partials);
            # fp16 pair pattern memset via a uint32 view halves the work
            acc = cpool.tile([128, N], F16, tag="acc")
            bits = int(np.float16(BIG).view(np.uint16))
            nc.vector._memset_packed(acc[:].bitcast(mybir.dt.uint32),
                                     bits | (bits << 16))

            strip_a = cpool.tile([128, NT], F32, tag="stripa")
            strip_b = cpool.tile([128, NT], F32, tag="stripb")
            t2strip = cpool.tile([128, 2], F32, tag="t2strip")

            def finish_group(g):
                # transpose TG acc tiles, min-reduce over the original lanes
                tp = tppool.tile([128, TG * 128], F16, tag="tp")
                for j in range(TG):
                    t = g * TG + j
                    nc.tensor.transpose(
                        out=tp[:, j * 128:(j + 1) * 128],
                        in_=acc[:, t * 128:(t + 1) * 128],
                        identity=eye_t[:])
                nc.vector.tensor_reduce(
                    out=strip_b[:, g * TG:(g + 1) * TG],
                    in_=tp[:].rearrange("p (t x) -> p t x", t=TG),
                    axis=mybir.AxisListType.X, op=MIN)

            # acc cols [1024g, 1024(g+1)) are final once unit 8g+9 has run
            done_after = {min(8 * g + 9, NT - 1): g for g in range(NG)}
            done_after[NT - 1] = NG - 1

            # ---- tier 1 ----
            for i in range(NT):
                s0 = _win_start(i)
                ck = pspool.tile([128, W1], F32, tag="ps")
                nc.tensor.matmul(
                    out=ck[:],
                    lhsT=wa_t[:, i * 128:(i + 1) * 128],
                    rhs=mb_t[:, s0:s0 + W1],
                    start=True, stop=True)
                sk = sbpool.tile([128, W1], F16, tag="sc")
                nc.scalar.copy(out=sk[:], in_=ck[:])
                # a->b row-min: one scan consumes both halves; final state
                # lands in the strip cell via a stride-0 broadcast output
                nc.vector.tensor_tensor_scan(
                    out=strip_a[:, i:i + 1].broadcast_to([128, W1 // 2]),
                    data0=sk[:, 0:W1 // 2],
                    data1=sk[:, W1 // 2:W1],
                    initial=BIG, op0=MIN, op1=MIN)
                # b->a per-lane column mins (2x mode)
                nc.vector.tensor_tensor(
                    out=acc[:, s0:s0 + W1], in0=sk[:],
                    in1=acc[:, s0:s0 + W1], op=MIN)
                g = done_after.get(i)
                if g is not None and 8 * g + 9 <= NT - 1:
                    finish_group(g)
                elif i == NT - 1:
                    for gg in range(NG):
                        if 8 * gg + 9 > NT - 1:
                            finish_group(gg)

            # ---- tier 2: 128 gathered queries x all 8192, per direction ----
            for d, (w2_t, m_t) in enumerate(((w2a_t, mb_t), (w2b_t, ma_t))):
                cell = t2strip[:, d:d + 1]
                for q in range(N // (2 * T2C)):
                    ck = ps2pool.tile([128, 2 * T2C], F32, tag="ps2")
                    for h in range(2):
                        nc.tensor.matmul(
                            out=ck[:, h * T2C:(h + 1) * T2C],
                            lhsT=w2_t[:],
                            rhs=m_t[:, q * 2 * T2C + h * T2C:
                                    q * 2 * T2C + (h + 1) * T2C],
                            start=True, stop=True)
                    sk = sb2pool.tile([128, 2 * T2C], F16, tag="sc2")
                    nc.scalar.copy(out=sk[:], in_=ck[:])
                    nc.vector.tensor_tensor_scan(
                        out=cell.broadcast_to([128, T2C]),
                        data0=sk[:, 0:T2C],
                        data1=sk[:, T2C:2 * T2C],
                        initial=(BIG if q == 0 else cell),
                        op0=MIN, op1=MIN)

            nc.sync.dma_start(out=sa_d[:], in_=strip_a[:])
            nc.sync.dma_start(out=sb_d[:], in_=strip_b[:])
            nc.sync.dma_start(out=t2_d[0:1, :], in_=t2strip[:, 0:1])
            nc.sync.dma_start(out=t2_d[1:2, :], in_=t2strip[:, 1:2])
    nc.compile()
    return nc


def _get_nc():
    global _NC_CACHE
    if _NC_CACHE is None:
        _NC_CACHE = _build_nc()
    return _NC_CACHE


def _probe_rub(q_s: np.ndarray, c_s: np.ndarray) -> np.ndarray:
    """Certified upper bound on each sorted query's NN distance: min dist
    to the +-KPROBE rank-neighbours in the sorted candidate set."""
    n, m = len(q_s), len(c_s)
    pos = np.searchsorted(c_s[:, 0], q_s[:, 0]).astype(np.int64)
    # gather a [n, 2*KPROBE] window of candidate indices (clamped)
    base = np.clip(pos - KPROBE, 0, m - 2 * KPROBE)
    idx = base[:, None] + np.arange(2 * KPROBE)[None, :]
    cand = c_s[idx]                                   # [n, 2K, 3]
    dd = ((q_s[:, None, :] - cand) ** 2).sum(-1).min(axis=1)
    return np.sqrt(dd) * (1 + 1e-6) + 1e-9


def _unsafe_sets(a_s: np.ndarray, b_s: np.ndarray):
    """Indices (in sorted order) of points whose certified candidate
    interval exceeds their tier-1 coverage."""
    rua = _probe_rub(a_s, b_s)
    rub = _probe_rub(b_s, a_s)
    alo = np.searchsorted(b_s[:, 0], a_s[:, 0] - rua)
    ahi = np.searchsorted(b_s[:, 0], a_s[:, 0] + rua)
    blo = np.searchsorted(a_s[:, 0], b_s[:, 0] - rub)
    bhi = np.searchsorted(a_s[:, 0], b_s[:, 0] + rub)
    starts = np.array([_win_start(i) for i in range(NT)])
    ws = starts[np.arange(N) // 128]
    unsafe_a = np.nonzero((alo < ws) | (ahi > ws + W1))[0]
    cov_lo = np.full(N, N, dtype=np.int64)
    cov_hi = np.zeros(N, dtype=np.int64)
    for i in range(NT):
        s = starts[i]
        cov_lo[s:s + W1] = np.minimum(cov_lo[s:s + W1], i * 128)
        cov_hi[s:s + W1] = np.maximum(cov_hi[s:s + W1], (i + 1) * 128)
    unsafe_b = np.nonzero((blo < cov_lo) | (bhi > cov_hi))[0]
    return unsafe_a, unsafe_b


def _in_maps(array1: np.ndarray, array2: np.ndarray):
    eye = np.eye(128, dtype=np.float32)
    in_maps = []
    meta = []
    for c in range(B):
        a_s = array1[c][np.argsort(array1[c][:, 0], kind="stable")]
        b_s = array2[c][np.argsort(array2[c][:, 0], kind="stable")]
        ua, ub = _unsafe_sets(a_s, b_s)
        assert len(ua) <= T2 and len(ub) <= T2, (len(ua), len(ub))
        wa = _w_side(a_s)
        wb = _w_side(b_s)
        w2a = np.zeros((K, T2), dtype=BF)
        w2b = np.zeros((K, T2), dtype=BF)
        w2a[:, :len(ua)] = wa[:, ua]
        w2b[:, :len(ub)] = wb[:, ub]
        in_maps.append({"wa": wa, "mb": _m_side(b_s), "ma": _m_side(a_s),
                        "w2a": w2a, "w2b": w2b, "eye": eye})
        meta.append((ua, ub))
    return in_maps, meta


def kernel(array1: np.ndarray, array2: np.ndarray) -> np.ndarray:
    array1 = np.asarray(array1, dtype=np.float32)
    array2 = np.asarray(array2, dtype=np.float32)
    assert array1.shape == (B, N, 3) and array2.shape == (B, N, 3)

    in_maps, meta = _in_maps(array1, array2)
    nc = _get_nc()
    res = run_bass_kernel_spmd(nc, in_maps, list(range(B))).results

    s1 = 0.0
    s2 = 0.0
    for c in range(B):
        ua, ub = meta[c]
        mina = res[c]["sa"].astype(np.float64).T.reshape(-1)  # [N] by rank
        minb = res[c]["sb"].astype(np.float64).T.reshape(-1)
        t2v = res[c]["t2"].astype(np.float64)
        mina[ua] = t2v[0, :len(ua)]
        minb[ub] = t2v[1, :len(ub)]
        s1 += np.sqrt(np.maximum(mina, 0.0)).sum()
        s2 += np.sqrt(np.maximum(minb, 0.0)).sum()
    val = 0.5 * (s1 / (B * N) + s2 / (B * N))
    return np.float32(val)


# revision 14
# speedup vs baseline: 1.0682x; 1.0682x over previous
"""Chamfer loss on 8 Trainium2 NeuronCores — windowed candidates + exact
tier-2 fallback.

Data-parallel over batch B=8: core c handles batch element c.

Host preprocessing (per batch element): sort both point sets by their
x-coordinate.  The x-gap lower-bounds the Euclidean distance, so a
query's nearest neighbour lies within an x-rank window whose width
scales with its NN distance.  A cheap probe (distance to the +-128 rank
neighbours, O(N*256) host flops) yields a certified upper bound r_ub on
every point's NN distance, hence a certified candidate rank interval
[x - r_ub, x + r_ub].  Points whose interval fits their block's fixed
512-wide tier-1 window (>99% of points) are exactly solved by tier 1;
the few dozen others are exactly solved by a padded 128-query tier-2
full scan per direction.  The result is exact up to fp16 rounding of
individual distances.

Device algorithm (per core), all on NEGATED distances so that every
reduction is a MAX (the GPSIMD cross-lane reduce only supports max):
  tier 1: for each of 64 query blocks (128 sorted a-points), ONE K=24
  bf16 matmul (fp32 coords split into 3 bf16 components each: 6
  cross-product rows per coordinate + 3 rows per squared norm keep
  ~1e-7 absolute accuracy at full PE rate) produces the [128, 512]
  -d tile in PSUM.  ScalarE converts it to fp16 in SBUF.  VectorE
  row-max-reduces it with one tensor_tensor_scan (max,max, stride-0
  broadcast output) -> a->b minima, and folds it into a persistent
  per-lane column-max accumulator with one 2x-mode tensor_tensor max
  -> b->a partial minima.  Each distance costs one ScalarE touch and
  one DVE cycle.  The accumulator [128, 8192] is finished on the
  otherwise-idle GPSIMD engine (partition_all_reduce max per 1024-col
  group, interleaved as regions become final) or, as fallback, by PE
  transposes + DVE reduces.
  tier 2: 128 gathered queries per direction; 16 matmul chunks whose
  fp16 copies fold into per-direction accumulators with 2x tensor_
  tensor maxes, finished by one scan each.
Per-point minima ship to the host, which substitutes tier-2 values for
the flagged points and does relu/sqrt/mean in fp64.
"""

import numpy as np
import ml_dtypes

import concourse.bass as bass
import concourse.mybir as mybir
import concourse.tile as tile
from concourse import bacc, bass_isa
from concourse.bass_utils import run_bass_kernel_spmd

B = 8
N = 8192          # points per set
K = 24            # augmented contraction rows
NT = N // 128     # 64 blocks of 128 sorted points
W1 = 512          # tier-1 candidate window per query block
T2 = 128          # tier-2 queries per direction (padded)
T2C = 1024        # tier-2 candidate chunk (2 matmuls of 512)
TG = 8            # accumulator tiles per finish group
KPROBE = 128      # host probe: +-KPROBE rank neighbours bound the NN dist
GPS_FINISH = True  # finish b->a on GPSIMD (False: PE transpose + DVE)
F32 = mybir.dt.float32
F16 = mybir.dt.float16
BF16 = mybir.dt.bfloat16
BF = ml_dtypes.bfloat16
NEGBIG = -60000.0  # fp16-safe "-infinity" (distances are negated)

_NC_CACHE = None


def _split3(v32: np.ndarray):
    """fp32 -> (hi, mid, lo) bf16 triple with hi+mid+lo == v to ~2^-24 rel."""
    v1 = v32.astype(BF)
    r = v32 - v1.astype(np.float32)
    v2 = r.astype(BF)
    v3 = (r - v2.astype(np.float32)).astype(BF)
    return v1, v2, v3


def _w_side(pts: np.ndarray):
    """pts [n,3] fp32 -> w [24,n] bf16 stationary-side operand, NEGATED so
    the matmul yields -squared-distance.

    Row pairing (per coordinate k, g = split3(+2*coord), h = split3(coord)):
      w rows: g1 g1 g2 g2 g1 g3   (m rows: h1 h2 h1 h2 h3 h1)
    so sum_r w[r]*m[r] = +2*coord_a*coord_b up to ~2^-26 terms.
    Rows 18-20: split3(-||.||^2) against ones; rows 21-23: -ones against
    the other side's split3(||.||^2).
    """
    s = -(pts.astype(np.float64) ** 2).sum(axis=1).astype(np.float32)
    s1, s2, s3 = _split3(s)
    w = np.empty((K, pts.shape[0]), dtype=BF)
    for k in range(3):
        c = pts[:, k].astype(np.float32)
        g1, g2, g3 = _split3(2.0 * c)
        r = 6 * k
        w[r + 0], w[r + 1], w[r + 2] = g1, g1, g2
        w[r + 3], w[r + 4], w[r + 5] = g2, g1, g3
    one = np.ones(pts.shape[0], dtype=BF)
    w[18], w[19], w[20] = s1, s2, s3
    w[21], w[22], w[23] = -one, -one, -one
    return w


def _m_side(pts: np.ndarray):
    """pts [n,3] fp32 -> m [24,n] bf16 moving-side operand (see _w_side)."""
    s = (pts.astype(np.float64) ** 2).sum(axis=1).astype(np.float32)
    s1, s2, s3 = _split3(s)
    m = np.empty((K, pts.shape[0]), dtype=BF)
    for k in range(3):
        c = pts[:, k].astype(np.float32)
        h1, h2, h3 = _split3(c)
        r = 6 * k
        m[r + 0], m[r + 1], m[r + 2] = h1, h2, h1
        m[r + 3], m[r + 4], m[r + 5] = h2, h3, h1
    one = np.ones(pts.shape[0], dtype=BF)
    m[18], m[19], m[20] = one, one, one
    m[21], m[22], m[23] = s1, s2, s3
    return m


def _win_start(i: int) -> int:
    """Tier-1 window start (rank-centred on block i, clamped)."""
    return min(max(i * 128 + 64 - W1 // 2, 0), N - W1)


def _build_nc():
    nc = bacc.Bacc(None)
    wa_d = nc.declare_dram_parameter("wa", [K, N], BF16, isOutput=False)
    mb_d = nc.declare_dram_parameter("mb", [K, N], BF16, isOutput=False)
    ma_d = nc.declare_dram_parameter("ma", [K, N], BF16, isOutput=False)
    w2a_d = nc.declare_dram_parameter("w2a", [K, T2], BF16, isOutput=False)
    w2b_d = nc.declare_dram_parameter("w2b", [K, T2], BF16, isOutput=False)
    sa_d = nc.declare_dram_parameter("sa", [128, NT], F32, isOutput=True)
    if GPS_FINISH:
        sb_d = nc.declare_dram_parameter("sb", [1, N], F32, isOutput=True)
    else:
        sb_d = nc.declare_dram_parameter("sb", [128, NT], F32, isOutput=True)
        eye_d = nc.declare_dram_parameter("eye", [128, 128], F32,
                                          isOutput=False)
    t2_d = nc.declare_dram_parameter("t2", [2, 128], F32, isOutput=True)

    MAX = mybir.AluOpType.max
    NG = NT // TG  # finish groups

    with tile.TileContext(nc) as tc:
        with (
            tc.tile_pool(name="const", bufs=1) as cpool,
            tc.tile_pool(name="psum", bufs=3, space="PSUM") as pspool,
            tc.tile_pool(name="psum2", bufs=2, space="PSUM") as ps2pool,
            tc.tile_pool(name="tpsum", bufs=1, space="PSUM") as tppool,
            tc.tile_pool(name="scopy", bufs=4) as sbpool,
            tc.tile_pool(name="scopy2", bufs=2) as sb2pool,
            tc.tile_pool(name="par", bufs=2) as parpool,
        ):
            wa_t = cpool.tile([K, N], BF16, tag="wa")
            mb_t = cpool.tile([K, N], BF16, tag="mb")
            ma_t = cpool.tile([K, N], BF16, tag="ma")
            w2a_t = cpool.tile([K, T2], BF16, tag="w2a")
            w2b_t = cpool.tile([K, T2], BF16, tag="w2b")
            # first tier-1 unit only needs the head of wa/mb: land those first
            nc.sync.dma_start(out=wa_t[:, 0:1024], in_=wa_d[:, 0:1024])
            nc.sync.dma_start(out=mb_t[:, 0:1024], in_=mb_d[:, 0:1024])
            for t, dram in ((wa_t, wa_d), (mb_t, mb_d)):
                for h in range(1, 8):
                    nc.sync.dma_start(out=t[:, h * 1024:(h + 1) * 1024],
                                      in_=dram[:, h * 1024:(h + 1) * 1024])
            nc.sync.dma_start(out=ma_t[:], in_=ma_d[:])
            nc.sync.dma_start(out=w2a_t[:], in_=w2a_d[:])
            nc.sync.dma_start(out=w2b_t[:], in_=w2b_d[:])
            if not GPS_FINISH:
                eyef_t = cpool.tile([128, 128], F32, tag="eyef")
                eye_t = cpool.tile([128, 128], F16, tag="eye")
                nc.sync.dma_start(out=eyef_t[:], in_=eye_d[:])
                nc.scalar.copy(out=eye_t[:], in_=eyef_t[:])

            # persistent per-lane column-max accumulator (b->a partials)
            acc = cpool.tile([128, N], F16, tag="acc")
            bits = int(np.float16(NEGBIG).view(np.uint16))
            nc.vector._memset_packed(acc[:].bitcast(mybir.dt.uint32),
                                     bits | (bits << 16))
            # tier-2 per-direction accumulators
            t2acc = cpool.tile([128, 2 * T2C], F16, tag="t2acc")
            nc.vector._memset_packed(t2acc[:].bitcast(mybir.dt.uint32),
                                     bits | (bits << 16))

            strip_a = cpool.tile([128, NT], F32, tag="stripa")
            strip_b = (None if GPS_FINISH else
                       cpool.tile([128, NT], F32, tag="stripb"))
            t2strip = cpool.tile([128, 2], F32, tag="t2strip")

            def finish_group(g):
                if GPS_FINISH:
                    par = parpool.tile([128, TG * 128], F32, tag="par")
                    nc.gpsimd.partition_all_reduce(
                        par[:], acc[:, g * TG * 128:(g + 1) * TG * 128],
                        channels=128, reduce_op=bass_isa.ReduceOp.max)
                    nc.sync.dma_start(
                        out=sb_d[0:1, g * TG * 128:(g + 1) * TG * 128],
                        in_=par[0:1, :])
                else:
                    tp = tppool.tile([128, TG * 128], F16, tag="tp")
                    for j in range(TG):
                        t = g * TG + j
                        nc.tensor.transpose(
                            out=tp[:, j * 128:(j + 1) * 128],
                            in_=acc[:, t * 128:(t + 1) * 128],
                            identity=eye_t[:])
                    nc.vector.tensor_reduce(
                        out=strip_b[:, g * TG:(g + 1) * TG],
                        in_=tp[:].rearrange("p (t x) -> p t x", t=TG),
                        axis=mybir.AxisListType.X, op=MAX)

            # acc cols [1024g, 1024(g+1)) are final once unit 8g+9 has run
            fin_after = {}
            for g in range(NG):
                fin_after.setdefault(min(8 * g + 9, NT - 1), []).append(g)

            # tier-2 chunk schedule: chunk (d, q) after tier-1 unit 4*(8d+q)+2
            t2_at = {4 * (8 * d + q) + 2: (d, q)
                     for d in range(2) for q in range(8)}

            def t2_chunk(d, q):
                w2_t, m_t = ((w2a_t, mb_t), (w2b_t, ma_t))[d]
                ck = ps2pool.tile([128, T2C], F32, tag="ps2")
                for h in range(2):
                    nc.tensor.matmul(
                        out=ck[:, h * 512:(h + 1) * 512],
                        lhsT=w2_t[:],
                        rhs=m_t[:, q * T2C + h * 512:q * T2C + (h + 1) * 512],
                        start=True, stop=True)
                sk = sb2pool.tile([128, T2C], F16, tag="sc2")
                nc.scalar.copy(out=sk[:], in_=ck[:])
                lane = t2acc[:, d * T2C:(d + 1) * T2C]
                nc.vector.tensor_tensor(out=lane, in0=sk[:], in1=lane, op=MAX)

            # ---- tier 1 (with tier-2 chunks and finish groups woven in) ----
            for i in range(NT):
                s0 = _win_start(i)
                ck = pspool.tile([128, W1], F32, tag="ps")
                nc.tensor.matmul(
                    out=ck[:],
                    lhsT=wa_t[:, i * 128:(i + 1) * 128],
                    rhs=mb_t[:, s0:s0 + W1],
                    start=True, stop=True)
                sk = sbpool.tile([128, W1], F16, tag="sc")
                nc.scalar.copy(out=sk[:], in_=ck[:])
                # a->b row-max: one scan consumes both halves; final state
                # lands in the strip cell via a stride-0 broadcast output
                nc.vector.tensor_tensor_scan(
                    out=strip_a[:, i:i + 1].broadcast_to([128, W1 // 2]),
                    data0=sk[:, 0:W1 // 2],
                    data1=sk[:, W1 // 2:W1],
                    initial=NEGBIG, op0=MAX, op1=MAX)
                # b->a per-lane column maxes (2x mode)
                nc.vector.tensor_tensor(
                    out=acc[:, s0:s0 + W1], in0=sk[:],
                    in1=acc[:, s0:s0 + W1], op=MAX)
                if i in t2_at:
                    t2_chunk(*t2_at[i])
                for g in fin_after.get(i, ()):
                    finish_group(g)

            # tier-2 finals
            for d in range(2):
                lane = t2acc[:, d * T2C:(d + 1) * T2C]
                nc.vector.tensor_tensor_scan(
                    out=t2strip[:, d:d + 1].broadcast_to([128, T2C // 2]),
                    data0=lane[:, 0:T2C // 2],
                    data1=lane[:, T2C // 2:T2C],
                    initial=NEGBIG, op0=MAX, op1=MAX)

            nc.sync.dma_start(out=sa_d[:], in_=strip_a[:])
            if not GPS_FINISH:
                nc.sync.dma_start(out=sb_d[:], in_=strip_b[:])
            nc.sync.dma_start(out=t2_d[0:1, :], in_=t2strip[:, 0:1])
            nc.sync.dma_start(out=t2_d[1:2, :], in_=t2strip[:, 1:2])
    nc.compile()
    return nc


def _get_nc():
    global _NC_CACHE
    if _NC_CACHE is None:
        _NC_CACHE = _build_nc()
    return _NC_CACHE


def _probe_rub(q_s: np.ndarray, c_s: np.ndarray) -> np.ndarray:
    """Certified upper bound on each sorted query's NN distance: min dist
    to the +-KPROBE rank-neighbours in the sorted candidate set."""
    n, m = len(q_s), len(c_s)
    pos = np.searchsorted(c_s[:, 0], q_s[:, 0]).astype(np.int64)
    base = np.clip(pos - KPROBE, 0, m - 2 * KPROBE)
    idx = base[:, None] + np.arange(2 * KPROBE)[None, :]
    cand = c_s[idx]                                   # [n, 2K, 3]
    dd = ((q_s[:, None, :] - cand) ** 2).sum(-1).min(axis=1)
    return np.sqrt(dd) * (1 + 1e-6) + 1e-9


def _unsafe_sets(a_s: np.ndarray, b_s: np.ndarray):
    """Indices (in sorted order) of points whose certified candidate
    interval exceeds their tier-1 coverage."""
    rua = _probe_rub(a_s, b_s)
    rub = _probe_rub(b_s, a_s)
    alo = np.searchsorted(b_s[:, 0], a_s[:, 0] - rua)
    ahi = np.searchsorted(b_s[:, 0], a_s[:, 0] + rua)
    blo = np.searchsorted(a_s[:, 0], b_s[:, 0] - rub)
    bhi = np.searchsorted(a_s[:, 0], b_s[:, 0] + rub)
    starts = np.array([_win_start(i) for i in range(NT)])
    ws = starts[np.arange(N) // 128]
    unsafe_a = np.nonzero((alo < ws) | (ahi > ws + W1))[0]
    cov_lo = np.full(N, N, dtype=np.int64)
    cov_hi = np.zeros(N, dtype=np.int64)
    for i in range(NT):
        s = starts[i]
        cov_lo[s:s + W1] = np.minimum(cov_lo[s:s + W1], i * 128)
        cov_hi[s:s + W1] = np.maximum(cov_hi[s:s + W1], (i + 1) * 128)
    unsafe_b = np.nonzero((blo < cov_lo) | (bhi > cov_hi))[0]
    return unsafe_a, unsafe_b


def _in_maps(array1: np.ndarray, array2: np.ndarray):
    in_maps = []
    meta = []
    eye = np.eye(128, dtype=np.float32)
    for c in range(B):
        a_s = array1[c][np.argsort(array1[c][:, 0], kind="stable")]
        b_s = array2[c][np.argsort(array2[c][:, 0], kind="stable")]
        ua, ub = _unsafe_sets(a_s, b_s)
        assert len(ua) <= T2 and len(ub) <= T2, (len(ua), len(ub))
        wa = _w_side(a_s)
        wb = _w_side(b_s)
        w2a = np.zeros((K, T2), dtype=BF)
        w2b = np.zeros((K, T2), dtype=BF)
        w2a[:, :len(ua)] = wa[:, ua]
        w2b[:, :len(ub)] = wb[:, ub]
        im = {"wa": wa, "mb": _m_side(b_s), "ma": _m_side(a_s),
              "w2a": w2a, "w2b": w2b}
        if not GPS_FINISH:
            im["eye"] = eye
        in_maps.append(im)
        meta.append((ua, ub))
    return in_maps, meta


def kernel(array1: np.ndarray, array2: np.ndarray) -> np.ndarray:
    array1 = np.asarray(array1, dtype=np.float32)
    array2 = np.asarray(array2, dtype=np.float32)
    assert array1.shape == (B, N, 3) and array2.shape == (B, N, 3)

    in_maps, meta = _in_maps(array1, array2)
    nc = _get_nc()
    res = run_bass_kernel_spmd(nc, in_maps, list(range(B))).results

    s1 = 0.0
    s2 = 0.0
    for c in range(B):
        ua, ub = meta[c]
        mina = -res[c]["sa"].astype(np.float64).T.reshape(-1)  # [N] by rank
        if GPS_FINISH:
            minb = -res[c]["sb"].astype(np.float64).reshape(-1)
        else:
            minb = -res[c]["sb"].astype(np.float64).T.reshape(-1)
        t2v = -res[c]["t2"].astype(np.float64)
        mina[ua] = t2v[0, :len(ua)]
        minb[ub] = t2v[1, :len(ub)]
        s1 += np.sqrt(np.maximum(mina, 0.0)).sum()
        s2 += np.sqrt(np.maximum(minb, 0.0)).sum()
    val = 0.5 * (s1 / (B * N) + s2 / (B * N))
    return np.float32(val)


# revision 22
# speedup vs baseline: 1.1245x; 1.0527x over previous
"""Chamfer loss on 8 Trainium2 NeuronCores — windowed candidates + exact
tier-2 fallback.

Data-parallel over batch B=8: core c handles batch element c.

Host preprocessing (per batch element): sort both point sets by their
x-coordinate.  The x-gap lower-bounds the Euclidean distance, so a
query's nearest neighbour lies within an x-rank window whose width
scales with its NN distance.  A cheap probe (distance to the +-128 rank
neighbours, O(N*256) host flops) yields a certified upper bound r_ub on
every point's NN distance, hence a certified candidate rank interval
[x - r_ub, x + r_ub].  Points whose interval fits their block's fixed
512-wide tier-1 window (>99% of points) are exactly solved by tier 1;
the few dozen others are exactly solved by a padded 128-query tier-2
full scan per direction.  The result is exact up to fp16 rounding of
individual distances.

Device algorithm (per core), all on NEGATED distances so that every
reduction is a MAX (the GPSIMD cross-lane reduce only supports max):
  tier 1: for each of 64 query blocks (128 sorted a-points), ONE K=24
  bf16 matmul (fp32 coords split into 3 bf16 components each: 6
  cross-product rows per coordinate + 3 rows per squared norm keep
  ~1e-7 absolute accuracy at full PE rate) produces the [128, 512]
  -d tile in PSUM.  ScalarE converts it to fp16 in SBUF.  VectorE
  row-max-reduces it with one tensor_tensor_scan (max,max, stride-0
  broadcast output) -> a->b minima, and folds it into a persistent
  per-lane column-max accumulator with one 2x-mode tensor_tensor max
  -> b->a partial minima.  Each distance costs one ScalarE touch and
  one DVE cycle.  The accumulator [128, 8192] is finished on the
  otherwise-idle GPSIMD engine (partition_all_reduce max per 1024-col
  group, interleaved as regions become final) or, as fallback, by PE
  transposes + DVE reduces.
  tier 2: 128 gathered queries per direction; 16 matmul chunks whose
  fp16 copies fold into per-direction accumulators with 2x tensor_
  tensor maxes, finished by one scan each.
Per-point minima ship to the host, which substitutes tier-2 values for
the flagged points and does relu/sqrt/mean in fp64.
"""

import numpy as np
import ml_dtypes

import concourse.bass as bass
import concourse.mybir as mybir
import concourse.tile as tile
from concourse import bacc, bass_isa
from concourse.bass_utils import run_bass_kernel_spmd

B = 8
N = 8192          # points per set
K = 24            # augmented contraction rows
NT = N // 128     # 64 blocks of 128 sorted points
W1 = 512          # tier-1 max candidate window per query block
T2 = 128          # tier-2 queries per direction (padded)
T2C = 1024        # tier-2 candidate chunk (2 matmuls of 512)
TG = 8            # accumulator tiles per finish group
KPROBE = 192      # host probe: +-KPROBE rank neighbours bound the NN dist
GPS_FINISH = True  # finish b->a on GPSIMD (False: PE transpose + DVE)
F32 = mybir.dt.float32
F16 = mybir.dt.float16
BF16 = mybir.dt.bfloat16
BF = ml_dtypes.bfloat16
NEGBIG = -60000.0  # fp16-safe "-infinity" (distances are negated)

_NC_CACHE = None


def _split3(v32: np.ndarray):
    """fp32 -> (hi, mid, lo) bf16 triple with hi+mid+lo == v to ~2^-24 rel."""
    v1 = v32.astype(BF)
    r = v32 - v1.astype(np.float32)
    v2 = r.astype(BF)
    v3 = (r - v2.astype(np.float32)).astype(BF)
    return v1, v2, v3


def _w_side(pts: np.ndarray):
    """pts [n,3] fp32 -> w [24,n] bf16 stationary-side operand, NEGATED so
    the matmul yields -squared-distance.

    Row pairing (per coordinate k, g = split3(+2*coord), h = split3(coord)):
      w rows: g1 g1 g2 g2 g1 g3   (m rows: h1 h2 h1 h2 h3 h1)
    so sum_r w[r]*m[r] = +2*coord_a*coord_b up to ~2^-26 terms.
    Rows 18-20: split3(-||.||^2) against ones; rows 21-23: -ones against
    the other side's split3(||.||^2).
    """
    s = -(pts.astype(np.float64) ** 2).sum(axis=1).astype(np.float32)
    s1, s2, s3 = _split3(s)
    w = np.empty((K, pts.shape[0]), dtype=BF)
    for k in range(3):
        c = pts[:, k].astype(np.float32)
        g1, g2, g3 = _split3(2.0 * c)
        r = 6 * k
        w[r + 0], w[r + 1], w[r + 2] = g1, g1, g2
        w[r + 3], w[r + 4], w[r + 5] = g2, g1, g3
    one = np.ones(pts.shape[0], dtype=BF)
    w[18], w[19], w[20] = s1, s2, s3
    w[21], w[22], w[23] = -one, -one, -one
    return w


def _m_side(pts: np.ndarray):
    """pts [n,3] fp32 -> m [24,n] bf16 moving-side operand (see _w_side)."""
    s = (pts.astype(np.float64) ** 2).sum(axis=1).astype(np.float32)
    s1, s2, s3 = _split3(s)
    m = np.empty((K, pts.shape[0]), dtype=BF)
    for k in range(3):
        c = pts[:, k].astype(np.float32)
        h1, h2, h3 = _split3(c)
        r = 6 * k
        m[r + 0], m[r + 1], m[r + 2] = h1, h2, h1
        m[r + 3], m[r + 4], m[r + 5] = h2, h3, h1
    one = np.ones(pts.shape[0], dtype=BF)
    m[18], m[19], m[20] = one, one, one
    m[21], m[22], m[23] = s1, s2, s3
    return m


def _win_start(i: int) -> int:
    """Baseline tier-1 window start (rank-centred on block i, clamped)."""
    return min(max(i * 128 + 64 - W1 // 2, 0), N - W1)


# per-block (start, width) table, set by _in_maps before _get_nc builds
_WINDOWS: list[tuple[int, int]] | None = None


def _build_nc():
    nc = bacc.Bacc(None)
    wa_d = nc.declare_dram_parameter("wa", [K, N], BF16, isOutput=False)
    mb_d = nc.declare_dram_parameter("mb", [K, N], BF16, isOutput=False)
    ma_d = nc.declare_dram_parameter("ma", [K, N], BF16, isOutput=False)
    w2a_d = nc.declare_dram_parameter("w2a", [K, T2], BF16, isOutput=False)
    w2b_d = nc.declare_dram_parameter("w2b", [K, T2], BF16, isOutput=False)
    sa_d = nc.declare_dram_parameter("sa", [128, NT], F32, isOutput=True)
    if GPS_FINISH:
        sb_d = nc.declare_dram_parameter("sb", [1, N], F32, isOutput=True)
    else:
        sb_d = nc.declare_dram_parameter("sb", [128, NT], F32, isOutput=True)
        eye_d = nc.declare_dram_parameter("eye", [128, 128], F32,
                                          isOutput=False)
    t2_d = nc.declare_dram_parameter("t2", [2, 128], F32, isOutput=True)

    MAX = mybir.AluOpType.max
    NG = NT // TG  # finish groups

    with tile.TileContext(nc) as tc:
        with (
            tc.tile_pool(name="const", bufs=1) as cpool,
            tc.tile_pool(name="psum", bufs=3, space="PSUM") as pspool,
            tc.tile_pool(name="psum2", bufs=2, space="PSUM") as ps2pool,
            tc.tile_pool(name="tpsum", bufs=1, space="PSUM") as tppool,
            tc.tile_pool(name="scopy", bufs=4) as sbpool,
            tc.tile_pool(name="scopy2", bufs=2) as sb2pool,
            tc.tile_pool(name="par", bufs=2) as parpool,
        ):
            wa_t = cpool.tile([K, N], BF16, tag="wa")
            mb_t = cpool.tile([K, N], BF16, tag="mb")
            ma_t = cpool.tile([K, N], BF16, tag="ma")
            w2a_t = cpool.tile([K, T2], BF16, tag="w2a")
            w2b_t = cpool.tile([K, T2], BF16, tag="w2b")
            # first tier-1 unit only needs the head of wa/mb: land those first
            nc.sync.dma_start(out=wa_t[:, 0:1024], in_=wa_d[:, 0:1024])
            nc.sync.dma_start(out=mb_t[:, 0:1024], in_=mb_d[:, 0:1024])
            for t, dram in ((wa_t, wa_d), (mb_t, mb_d)):
                for h in range(1, 8):
                    nc.sync.dma_start(out=t[:, h * 1024:(h + 1) * 1024],
                                      in_=dram[:, h * 1024:(h + 1) * 1024])
            nc.sync.dma_start(out=ma_t[:], in_=ma_d[:])
            nc.sync.dma_start(out=w2a_t[:], in_=w2a_d[:])
            nc.sync.dma_start(out=w2b_t[:], in_=w2b_d[:])
            if not GPS_FINISH:
                eyef_t = cpool.tile([128, 128], F32, tag="eyef")
                eye_t = cpool.tile([128, 128], F16, tag="eye")
                nc.sync.dma_start(out=eyef_t[:], in_=eye_d[:])
                nc.scalar.copy(out=eye_t[:], in_=eyef_t[:])

            # persistent per-lane column-max accumulator (b->a partials)
            acc = cpool.tile([128, N], F16, tag="acc")
            bits = int(np.float16(NEGBIG).view(np.uint16))
            nc.vector._memset_packed(acc[:].bitcast(mybir.dt.uint32),
                                     bits | (bits << 16))
            # tier-2 per-direction accumulators
            t2acc = cpool.tile([128, 2 * T2C], F16, tag="t2acc")
            nc.vector._memset_packed(t2acc[:].bitcast(mybir.dt.uint32),
                                     bits | (bits << 16))

            strip_a = cpool.tile([128, NT], F32, tag="stripa")
            strip_b = (None if GPS_FINISH else
                       cpool.tile([128, NT], F32, tag="stripb"))
            t2strip = cpool.tile([128, 2], F32, tag="t2strip")

            def finish_group(g):
                if GPS_FINISH:
                    par = parpool.tile([128, TG * 128], F32, tag="par")
                    nc.gpsimd.partition_all_reduce(
                        par[:], acc[:, g * TG * 128:(g + 1) * TG * 128],
                        channels=128, reduce_op=bass_isa.ReduceOp.max)
                    nc.sync.dma_start(
                        out=sb_d[0:1, g * TG * 128:(g + 1) * TG * 128],
                        in_=par[0:1, :])
                else:
                    tp = tppool.tile([128, TG * 128], F16, tag="tp")
                    for j in range(TG):
                        t = g * TG + j
                        nc.tensor.transpose(
                            out=tp[:, j * 128:(j + 1) * 128],
                            in_=acc[:, t * 128:(t + 1) * 128],
                            identity=eye_t[:])
                    nc.vector.tensor_reduce(
                        out=strip_b[:, g * TG:(g + 1) * TG],
                        in_=tp[:].rearrange("p (t x) -> p t x", t=TG),
                        axis=mybir.AxisListType.X, op=MAX)

            wins = (_WINDOWS if _WINDOWS is not None
                    else [(_win_start(i), W1) for i in range(NT)])

            # acc cols [1024g, 1024(g+1)) are final after the last unit
            # whose window starts below the boundary
            fin_after = {}
            for g in range(NG):
                bound = TG * 128 * (g + 1)
                i_fin = max(j for j in range(NT) if wins[j][0] < bound)
                fin_after.setdefault(i_fin, []).append(g)

            # tier-2 chunk schedule: chunk (d, q) after tier-1 unit 4*(8d+q)+2
            t2_at = {4 * (8 * d + q) + 2: (d, q)
                     for d in range(2) for q in range(8)}

            def t2_chunk(d, q):
                w2_t, m_t = ((w2a_t, mb_t), (w2b_t, ma_t))[d]
                ck = ps2pool.tile([128, T2C], F32, tag="ps2")
                for h in range(2):
                    nc.tensor.matmul(
                        out=ck[:, h * 512:(h + 1) * 512],
                        lhsT=w2_t[:],
                        rhs=m_t[:, q * T2C + h * 512:q * T2C + (h + 1) * 512],
                        start=True, stop=True)
                sk = sb2pool.tile([128, T2C], F16, tag="sc2")
                nc.scalar.copy(out=sk[:], in_=ck[:])
                lane = t2acc[:, d * T2C:(d + 1) * T2C]
                nc.vector.tensor_tensor(out=lane, in0=sk[:], in1=lane, op=MAX)

            # ---- tier 1 (with tier-2 chunks and finish groups woven in) ----
            for i in range(NT):
                s0, w = wins[i]
                ck = pspool.tile([128, W1], F32, tag="ps")
                nc.tensor.matmul(
                    out=ck[:, 0:w],
                    lhsT=wa_t[:, i * 128:(i + 1) * 128],
                    rhs=mb_t[:, s0:s0 + w],
                    start=True, stop=True)
                sk = sbpool.tile([128, W1], F16, tag="sc")
                nc.scalar.copy(out=sk[:, 0:w], in_=ck[:, 0:w])
                # b->a per-lane column maxes (2x mode); emitted before the
                # scan so the scan hides the acc write-ack latency that the
                # next unit's accumulator update must wait for
                nc.vector.tensor_tensor(
                    out=acc[:, s0:s0 + w], in0=sk[:, 0:w],
                    in1=acc[:, s0:s0 + w], op=MAX)
                # a->b row-max: one scan consumes both halves; final state
                # lands in the strip cell via a stride-0 broadcast output
                nc.vector.tensor_tensor_scan(
                    out=strip_a[:, i:i + 1].broadcast_to([128, w // 2]),
                    data0=sk[:, 0:w // 2],
                    data1=sk[:, w // 2:w],
                    initial=NEGBIG, op0=MAX, op1=MAX)
                if i in t2_at:
                    t2_chunk(*t2_at[i])
                for g in fin_after.get(i, ()):
                    finish_group(g)

            # tier-2 finals
            for d in range(2):
                lane = t2acc[:, d * T2C:(d + 1) * T2C]
                nc.vector.tensor_tensor_scan(
                    out=t2strip[:, d:d + 1].broadcast_to([128, T2C // 2]),
                    data0=lane[:, 0:T2C // 2],
                    data1=lane[:, T2C // 2:T2C],
                    initial=NEGBIG, op0=MAX, op1=MAX)

            nc.sync.dma_start(out=sa_d[:], in_=strip_a[:])
            if not GPS_FINISH:
                nc.sync.dma_start(out=sb_d[:], in_=strip_b[:])
            nc.sync.dma_start(out=t2_d[0:1, :], in_=t2strip[:, 0:1])
            nc.sync.dma_start(out=t2_d[1:2, :], in_=t2strip[:, 1:2])
    nc.compile()
    return nc


def _get_nc():
    global _NC_CACHE
    key = tuple(_WINDOWS) if _WINDOWS is not None else None
    if _NC_CACHE is None or _NC_CACHE[0] != key:
        _NC_CACHE = (key, _build_nc())
    return _NC_CACHE[1]


def _probe_rub(q_s: np.ndarray, c_s: np.ndarray) -> np.ndarray:
    """Certified upper bound on each sorted query's NN distance: min dist
    to the +-KPROBE rank-neighbours in the sorted candidate set."""
    n, m = len(q_s), len(c_s)
    pos = np.searchsorted(c_s[:, 0], q_s[:, 0]).astype(np.int64)
    base = np.clip(pos - KPROBE, 0, m - 2 * KPROBE)
    idx = base[:, None] + np.arange(2 * KPROBE)[None, :]
    cand = c_s[idx]                                   # [n, 2K, 3]
    dd = ((q_s[:, None, :] - cand) ** 2).sum(-1).min(axis=1)
    return np.sqrt(dd) * (1 + 1e-6) + 1e-9


def _unsafe_sets(intervals, wins):
    """Per-core indices (sorted order) of points whose certified candidate
    interval exceeds their tier-1 coverage under the window table."""
    alo, ahi, blo, bhi = intervals
    starts = np.array([w[0] for w in wins])
    ends = np.array([w[0] + w[1] for w in wins])
    ws = starts[np.arange(N) // 128]
    we = ends[np.arange(N) // 128]
    unsafe_a = np.nonzero((alo < ws) | (ahi > we))[0]
    cov_lo = np.full(N, N, dtype=np.int64)
    cov_hi = np.zeros(N, dtype=np.int64)
    for i in range(NT):
        s, e = starts[i], ends[i]
        cov_lo[s:e] = np.minimum(cov_lo[s:e], i * 128)
        cov_hi[s:e] = np.maximum(cov_hi[s:e], (i + 1) * 128)
    unsafe_b = np.nonzero((blo < cov_lo) | (bhi > cov_hi))[0]
    return unsafe_a, unsafe_b


def _intervals(a_s, b_s):
    rua = _probe_rub(a_s, b_s)
    rub = _probe_rub(b_s, a_s)
    alo = np.searchsorted(b_s[:, 0], a_s[:, 0] - rua)
    ahi = np.searchsorted(b_s[:, 0], a_s[:, 0] + rua)
    blo = np.searchsorted(a_s[:, 0], b_s[:, 0] - rub)
    bhi = np.searchsorted(a_s[:, 0], b_s[:, 0] + rub)
    return alo, ahi, blo, bhi


def _plan_windows(all_intervals):
    """Choose one per-block (start, width<=W1) table covering, for every
    core, all points that the fixed-W1 baseline covers.  Points left out
    fall to tier 2; the exact per-core unsafe sets are recomputed from the
    final table, so shrinkage never violates the exactness guarantee."""
    base = [(_win_start(i), W1) for i in range(NT)]
    blk = np.arange(N) // 128
    s_need = np.full(NT, N, dtype=np.int64)
    e_need = np.zeros(NT, dtype=np.int64)
    for alo, ahi, _, _ in all_intervals:
        bs = np.array([b[0] for b in base])[blk]
        safe = (alo >= bs) & (ahi <= bs + W1)
        for i in range(NT):
            sel = safe[blk == i]
            if sel.any():
                pts = np.nonzero(blk == i)[0][sel]
                s_need[i] = min(s_need[i], alo[pts].min())
                e_need[i] = max(e_need[i], ahi[pts].max())
    wins = []
    prev_s = 0
    for i in range(NT):
        if s_need[i] >= e_need[i]:
            s, w = base[i]
        else:
            w = min(W1, max(128, -((s_need[i] - e_need[i]) // 64) * 64))
            s = max(0, min(s_need[i] & ~1, N - w))
        s = max(s, prev_s)
        s = min(s, N - w)
        prev_s = s
        wins.append((int(s), int(w)))
    return wins


def _in_maps(array1: np.ndarray, array2: np.ndarray):
    global _WINDOWS
    sorted_pts = []
    all_intervals = []
    for c in range(B):
        a_s = array1[c][np.argsort(array1[c][:, 0], kind="stable")]
        b_s = array2[c][np.argsort(array2[c][:, 0], kind="stable")]
        sorted_pts.append((a_s, b_s))
        all_intervals.append(_intervals(a_s, b_s))

    wins = _plan_windows(all_intervals)
    # widen if any core's tier-2 set exceeds capacity
    for _ in range(4):
        counts = [max(len(u) for u in _unsafe_sets(iv, wins))
                  for iv in all_intervals]
        if max(counts) <= T2:
            break
        wins = [(max(0, min(s - 32, N - min(W1, w + 64))),
                 min(W1, w + 64)) for s, w in wins]
    _WINDOWS = wins

    in_maps = []
    meta = []
    eye = np.eye(128, dtype=np.float32)
    for c in range(B):
        a_s, b_s = sorted_pts[c]
        ua, ub = _unsafe_sets(all_intervals[c], wins)
        assert len(ua) <= T2 and len(ub) <= T2, (len(ua), len(ub))
        wa = _w_side(a_s)
        wb = _w_side(b_s)
        w2a = np.zeros((K, T2), dtype=BF)
        w2b = np.zeros((K, T2), dtype=BF)
        w2a[:, :len(ua)] = wa[:, ua]
        w2b[:, :len(ub)] = wb[:, ub]
        im = {"wa": wa, "mb": _m_side(b_s), "ma": _m_side(a_s),
              "w2a": w2a, "w2b": w2b}
        if not GPS_FINISH:
            im["eye"] = eye
        in_maps.append(im)
        meta.append((ua, ub))
    return in_maps, meta


def kernel(array1: np.ndarray, array2: np.ndarray) -> np.ndarray:
    array1 = np.asarray(array1, dtype=np.float32)
    array2 = np.asarray(array2, dtype=np.float32)
    assert array1.shape == (B, N, 3) and array2.shape == (B, N, 3)

    in_maps, meta = _in_maps(array1, array2)
    nc = _get_nc()
    res = run_bass_kernel_spmd(nc, in_maps, list(range(B))).results

    s1 = 0.0
    s2 = 0.0
    for c in range(B):
        ua, ub = meta[c]
        mina = -res[c]["sa"].astype(np.float64).T.reshape(-1)  # [N] by rank
        if GPS_FINISH:
            minb = -res[c]["sb"].astype(np.float64).reshape(-1)
        else:
            minb = -res[c]["sb"].astype(np.float64).T.reshape(-1)
        t2v = -res[c]["t2"].astype(np.float64)
        mina[ua] = t2v[0, :len(ua)]
        minb[ub] = t2v[1, :len(ub)]
        s1 += np.sqrt(np.maximum(mina, 0.0)).sum()
        s2 += np.sqrt(np.maximum(minb, 0.0)).sum()
    val = 0.5 * (s1 / (B * N) + s2 / (B * N))
    return np.float32(val)


# revision 27
# speedup vs baseline: 1.1988x; 1.0661x over previous
"""Chamfer loss on 8 Trainium2 NeuronCores — windowed candidates + exact
tier-2 fallback.

Data-parallel over batch B=8: core c handles batch element c.

Host preprocessing (per batch element): sort both point sets by their
x-coordinate.  The x-gap lower-bounds the Euclidean distance, so a
query's nearest neighbour lies within an x-rank window whose width
scales with its NN distance.  A cheap probe (distance to the +-128 rank
neighbours, O(N*256) host flops) yields a certified upper bound r_ub on
every point's NN distance, hence a certified candidate rank interval
[x - r_ub, x + r_ub].  Points whose interval fits their block's fixed
512-wide tier-1 window (>99% of points) are exactly solved by tier 1;
the few dozen others are exactly solved by a padded 128-query tier-2
full scan per direction.  The result is exact up to fp16 rounding of
individual distances.

Device algorithm (per core), all on NEGATED distances so that every
reduction is a MAX (the GPSIMD cross-lane reduce only supports max):
  tier 1: for each of 64 query blocks (128 sorted a-points), ONE K=24
  bf16 matmul (fp32 coords split into 3 bf16 components each: 6
  cross-product rows per coordinate + 3 rows per squared norm keep
  ~1e-7 absolute accuracy at full PE rate) produces the [128, 512]
  -d tile in PSUM.  ScalarE converts it to fp16 in SBUF.  VectorE
  row-max-reduces it with one tensor_tensor_scan (max,max, stride-0
  broadcast output) -> a->b minima, and folds it into a persistent
  per-lane column-max accumulator with one 2x-mode tensor_tensor max
  -> b->a partial minima.  Each distance costs one ScalarE touch and
  one DVE cycle.  The accumulator [128, 8192] is finished on the
  otherwise-idle GPSIMD engine (partition_all_reduce max per 1024-col
  group, interleaved as regions become final) or, as fallback, by PE
  transposes + DVE reduces.
  tier 2: 128 gathered queries per direction; 16 matmul chunks whose
  fp16 copies fold into per-direction accumulators with 2x tensor_
  tensor maxes, finished by one scan each.
Per-point minima ship to the host, which substitutes tier-2 values for
the flagged points and does relu/sqrt/mean in fp64.
"""

import numpy as np
import ml_dtypes

import concourse.bass as bass
import concourse.mybir as mybir
import concourse.tile as tile
from concourse import bacc, bass_isa
from concourse.bass_utils import run_bass_kernel_spmd

B = 8
N = 8192          # points per set
K = 24            # augmented contraction rows
NT = N // 128     # 64 blocks of 128 sorted points
W1 = 512          # tier-1 max candidate window per query block
T2 = 128          # tier-2 queries per direction (padded)
T2C = 1024        # tier-2 candidate chunk (2 matmuls of 512)
TG = 8            # accumulator tiles per finish group
KPROBE = 192      # host probe: +-KPROBE rank neighbours bound the NN dist
GPS_FINISH = True  # finish b->a on GPSIMD (False: PE transpose + DVE)
F32 = mybir.dt.float32
F16 = mybir.dt.float16
BF16 = mybir.dt.bfloat16
BF = ml_dtypes.bfloat16
NEGBIG = -60000.0  # fp16-safe "-infinity" (distances are negated)

_NC_CACHE = None


def _split3(v32: np.ndarray):
    """fp32 -> (hi, mid, lo) bf16 triple with hi+mid+lo == v to ~2^-24 rel."""
    v1 = v32.astype(BF)
    r = v32 - v1.astype(np.float32)
    v2 = r.astype(BF)
    v3 = (r - v2.astype(np.float32)).astype(BF)
    return v1, v2, v3


def _w_side(pts: np.ndarray):
    """pts [n,3] fp32 -> w [24,n] bf16 stationary-side operand, NEGATED so
    the matmul yields -squared-distance.

    Row pairing (per coordinate k, g = split3(+2*coord), h = split3(coord)):
      w rows: g1 g1 g2 g2 g1 g3   (m rows: h1 h2 h1 h2 h3 h1)
    so sum_r w[r]*m[r] = +2*coord_a*coord_b up to ~2^-26 terms.
    Rows 18-20: split3(-||.||^2) against ones; rows 21-23: -ones against
    the other side's split3(||.||^2).
    """
    s = -(pts.astype(np.float64) ** 2).sum(axis=1).astype(np.float32)
    s1, s2, s3 = _split3(s)
    w = np.empty((K, pts.shape[0]), dtype=BF)
    for k in range(3):
        c = pts[:, k].astype(np.float32)
        g1, g2, g3 = _split3(2.0 * c)
        r = 6 * k
        w[r + 0], w[r + 1], w[r + 2] = g1, g1, g2
        w[r + 3], w[r + 4], w[r + 5] = g2, g1, g3
    one = np.ones(pts.shape[0], dtype=BF)
    w[18], w[19], w[20] = s1, s2, s3
    w[21], w[22], w[23] = -one, -one, -one
    return w


def _m_side(pts: np.ndarray):
    """pts [n,3] fp32 -> m [24,n] bf16 moving-side operand (see _w_side)."""
    s = (pts.astype(np.float64) ** 2).sum(axis=1).astype(np.float32)
    s1, s2, s3 = _split3(s)
    m = np.empty((K, pts.shape[0]), dtype=BF)
    for k in range(3):
        c = pts[:, k].astype(np.float32)
        h1, h2, h3 = _split3(c)
        r = 6 * k
        m[r + 0], m[r + 1], m[r + 2] = h1, h2, h1
        m[r + 3], m[r + 4], m[r + 5] = h2, h3, h1
    one = np.ones(pts.shape[0], dtype=BF)
    m[18], m[19], m[20] = one, one, one
    m[21], m[22], m[23] = s1, s2, s3
    return m


def _win_start(i: int) -> int:
    """Baseline tier-1 window start (rank-centred on block i, clamped)."""
    return min(max(i * 128 + 64 - W1 // 2, 0), N - W1)


# per-block (start, width) table, set by _in_maps before _get_nc builds
_WINDOWS: list[tuple[int, int]] | None = None


def _build_nc():
    nc = bacc.Bacc(None)
    wa_d = nc.declare_dram_parameter("wa", [K, N], BF16, isOutput=False)
    mb_d = nc.declare_dram_parameter("mb", [K, N], BF16, isOutput=False)
    ma_d = nc.declare_dram_parameter("ma", [K, N], BF16, isOutput=False)
    w2a_d = nc.declare_dram_parameter("w2a", [K, T2], BF16, isOutput=False)
    w2b_d = nc.declare_dram_parameter("w2b", [K, T2], BF16, isOutput=False)
    sa_d = nc.declare_dram_parameter("sa", [128, NT], F32, isOutput=True)
    if GPS_FINISH:
        sb_d = nc.declare_dram_parameter("sb", [1, N], F32, isOutput=True)
    else:
        sb_d = nc.declare_dram_parameter("sb", [128, NT], F32, isOutput=True)
        eye_d = nc.declare_dram_parameter("eye", [128, 128], F32,
                                          isOutput=False)
    t2_d = nc.declare_dram_parameter("t2", [2, 128], F32, isOutput=True)

    MAX = mybir.AluOpType.max
    NG = NT // TG  # finish groups

    with tile.TileContext(nc) as tc:
        with (
            tc.tile_pool(name="const", bufs=1) as cpool,
            tc.tile_pool(name="psum", bufs=3, space="PSUM") as pspool,
            tc.tile_pool(name="psum2", bufs=2, space="PSUM") as ps2pool,
            tc.tile_pool(name="tpsum", bufs=1, space="PSUM") as tppool,
            tc.tile_pool(name="scopy", bufs=4) as sbpool,
            tc.tile_pool(name="scopy2", bufs=2) as sb2pool,
            tc.tile_pool(name="par", bufs=2) as parpool,
        ):
            wa_t = cpool.tile([K, N], BF16, tag="wa")
            mb_t = cpool.tile([K, N], BF16, tag="mb")
            ma_t = cpool.tile([K, N], BF16, tag="ma")
            w2a_t = cpool.tile([K, T2], BF16, tag="w2a")
            w2b_t = cpool.tile([K, T2], BF16, tag="w2b")
            # the first tier-1 units only need the head of wa/mb: land the
            # minimal slices first so the PE can start immediately
            nc.sync.dma_start(out=wa_t[:, 0:256], in_=wa_d[:, 0:256])
            nc.sync.dma_start(out=mb_t[:, 0:768], in_=mb_d[:, 0:768])
            nc.sync.dma_start(out=wa_t[:, 256:1024], in_=wa_d[:, 256:1024])
            nc.sync.dma_start(out=mb_t[:, 768:1536], in_=mb_d[:, 768:1536])
            for h in range(1, 8):
                nc.sync.dma_start(out=wa_t[:, h * 1024:(h + 1) * 1024],
                                  in_=wa_d[:, h * 1024:(h + 1) * 1024])
                lo, hi = 512 + h * 1024, min(512 + (h + 1) * 1024, N)
                nc.sync.dma_start(out=mb_t[:, lo:hi], in_=mb_d[:, lo:hi])
            nc.sync.dma_start(out=w2a_t[:], in_=w2a_d[:])
            nc.sync.dma_start(out=ma_t[:], in_=ma_d[:])
            nc.sync.dma_start(out=w2b_t[:], in_=w2b_d[:])
            if not GPS_FINISH:
                eyef_t = cpool.tile([128, 128], F32, tag="eyef")
                eye_t = cpool.tile([128, 128], F16, tag="eye")
                nc.sync.dma_start(out=eyef_t[:], in_=eye_d[:])
                nc.scalar.copy(out=eye_t[:], in_=eyef_t[:])

            # persistent per-lane column-max accumulator (b->a partials)
            acc = cpool.tile([128, N], F16, tag="acc")
            bits = int(np.float16(NEGBIG).view(np.uint16))
            nc.vector._memset_packed(acc[:].bitcast(mybir.dt.uint32),
                                     bits | (bits << 16))
            # tier-2 per-direction accumulators
            t2acc = cpool.tile([128, 2 * T2C], F16, tag="t2acc")
            nc.vector._memset_packed(t2acc[:].bitcast(mybir.dt.uint32),
                                     bits | (bits << 16))

            strip_a = cpool.tile([128, NT], F32, tag="stripa")
            strip_b = (None if GPS_FINISH else
                       cpool.tile([128, NT], F32, tag="stripb"))
            t2strip = cpool.tile([128, 2], F32, tag="t2strip")

            def finish_group(t0, t1):
                # b->a finish for acc tiles [t0, t1)
                if GPS_FINISH:
                    par = parpool.tile([128, TG * 128], F32, tag="par")
                    w = (t1 - t0) * 128
                    nc.gpsimd.partition_all_reduce(
                        par[:, 0:w], acc[:, t0 * 128:t1 * 128],
                        channels=128, reduce_op=bass_isa.ReduceOp.max)
                    nc.sync.dma_start(
                        out=sb_d[0:1, t0 * 128:t1 * 128],
                        in_=par[0:1, 0:w])
                else:
                    tp = tppool.tile([128, TG * 128], F16, tag="tp")
                    for j in range(t1 - t0):
                        t = t0 + j
                        nc.tensor.transpose(
                            out=tp[:, j * 128:(j + 1) * 128],
                            in_=acc[:, t * 128:(t + 1) * 128],
                            identity=eye_t[:])
                    nc.vector.tensor_reduce(
                        out=strip_b[:, t0:t1],
                        in_=tp[:, 0:(t1 - t0) * 128].rearrange(
                            "p (t x) -> p t x", t=t1 - t0),
                        axis=mybir.AxisListType.X, op=MAX)

            wins = (_WINDOWS if _WINDOWS is not None
                    else [(_win_start(i), W1) for i in range(NT)])

            # acc tile ranges are final after the last unit whose window
            # starts below the range end; keep the very last range small
            # so the closing partition-reduce is a short tail
            ranges = [(g * TG, (g + 1) * TG) for g in range(NG - 1)]
            ranges += [(NT - TG, NT - 4), (NT - 4, NT)]
            fin_after = {}
            for (t0, t1) in ranges:
                bound = 128 * t1
                i_fin = max(j for j in range(NT) if wins[j][0] < bound)
                fin_after.setdefault(i_fin, []).append((t0, t1))

            # tier-2 chunk schedule: chunk (d, q) after tier-1 unit 3*(8d+q)+2
            t2_at = {3 * (8 * d + q) + 2: (d, q)
                     for d in range(2) for q in range(8)}

            def t2_chunk(d, q):
                w2_t, m_t = ((w2a_t, mb_t), (w2b_t, ma_t))[d]
                ck = ps2pool.tile([128, T2C], F32, tag="ps2")
                for h in range(2):
                    nc.tensor.matmul(
                        out=ck[:, h * 512:(h + 1) * 512],
                        lhsT=w2_t[:],
                        rhs=m_t[:, q * T2C + h * 512:q * T2C + (h + 1) * 512],
                        start=True, stop=True)
                sk = sb2pool.tile([128, T2C], F16, tag="sc2")
                nc.scalar.copy(out=sk[:], in_=ck[:])
                lane = t2acc[:, d * T2C:(d + 1) * T2C]
                nc.vector.tensor_tensor(out=lane, in0=sk[:], in1=lane, op=MAX)

            # ---- tier 1 (with tier-2 chunks and finish groups woven in) ----
            for i in range(NT):
                s0, w = wins[i]
                ck = pspool.tile([128, W1], F32, tag="ps")
                nc.tensor.matmul(
                    out=ck[:, 0:w],
                    lhsT=wa_t[:, i * 128:(i + 1) * 128],
                    rhs=mb_t[:, s0:s0 + w],
                    start=True, stop=True)
                sk = sbpool.tile([128, W1], F16, tag="sc")
                nc.scalar.copy(out=sk[:, 0:w], in_=ck[:, 0:w])
                # b->a per-lane column maxes (2x mode); emitted before the
                # scan so the scan hides the acc write-ack latency that the
                # next unit's accumulator update must wait for
                nc.vector.tensor_tensor(
                    out=acc[:, s0:s0 + w], in0=sk[:, 0:w],
                    in1=acc[:, s0:s0 + w], op=MAX)
                # a->b row-max: one scan consumes both halves; final state
                # lands in the strip cell via a stride-0 broadcast output
                nc.vector.tensor_tensor_scan(
                    out=strip_a[:, i:i + 1].broadcast_to([128, w // 2]),
                    data0=sk[:, 0:w // 2],
                    data1=sk[:, w // 2:w],
                    initial=NEGBIG, op0=MAX, op1=MAX)
                if i in t2_at:
                    t2_chunk(*t2_at[i])
                for rng in fin_after.get(i, ()):
                    finish_group(*rng)
                if i == 50:
                    # tier-2 finals (all chunks landed by unit 47)
                    for d in range(2):
                        lane = t2acc[:, d * T2C:(d + 1) * T2C]
                        nc.vector.tensor_tensor_scan(
                            out=t2strip[:, d:d + 1].broadcast_to(
                                [128, T2C // 2]),
                            data0=lane[:, 0:T2C // 2],
                            data1=lane[:, T2C // 2:T2C],
                            initial=NEGBIG, op0=MAX, op1=MAX)
                    nc.sync.dma_start(out=t2_d[0:1, :], in_=t2strip[:, 0:1])
                    nc.sync.dma_start(out=t2_d[1:2, :], in_=t2strip[:, 1:2])
                if i % 16 == 15:
                    # ship finished strip columns early; only the last 16
                    # remain for the tail
                    nc.sync.dma_start(out=sa_d[:, i - 15:i + 1],
                                      in_=strip_a[:, i - 15:i + 1])

            if not GPS_FINISH:
                nc.sync.dma_start(out=sb_d[:], in_=strip_b[:])
    nc.compile()
    return nc


def _get_nc():
    global _NC_CACHE
    key = tuple(_WINDOWS) if _WINDOWS is not None else None
    if _NC_CACHE is None or _NC_CACHE[0] != key:
        _NC_CACHE = (key, _build_nc())
    return _NC_CACHE[1]


def _probe_rub(q_s: np.ndarray, c_s: np.ndarray) -> np.ndarray:
    """Certified upper bound on each sorted query's NN distance: min dist
    to the +-KPROBE rank-neighbours in the sorted candidate set."""
    n, m = len(q_s), len(c_s)
    pos = np.searchsorted(c_s[:, 0], q_s[:, 0]).astype(np.int64)
    base = np.clip(pos - KPROBE, 0, m - 2 * KPROBE)
    idx = base[:, None] + np.arange(2 * KPROBE)[None, :]
    cand = c_s[idx]                                   # [n, 2K, 3]
    dd = ((q_s[:, None, :] - cand) ** 2).sum(-1).min(axis=1)
    return np.sqrt(dd) * (1 + 1e-6) + 1e-9


def _unsafe_sets(intervals, wins):
    """Per-core indices (sorted order) of points whose certified candidate
    interval exceeds their tier-1 coverage under the window table."""
    alo, ahi, blo, bhi = intervals
    starts = np.array([w[0] for w in wins])
    ends = np.array([w[0] + w[1] for w in wins])
    ws = starts[np.arange(N) // 128]
    we = ends[np.arange(N) // 128]
    unsafe_a = np.nonzero((alo < ws) | (ahi > we))[0]
    cov_lo = np.full(N, N, dtype=np.int64)
    cov_hi = np.zeros(N, dtype=np.int64)
    for i in range(NT):
        s, e = starts[i], ends[i]
        cov_lo[s:e] = np.minimum(cov_lo[s:e], i * 128)
        cov_hi[s:e] = np.maximum(cov_hi[s:e], (i + 1) * 128)
    unsafe_b = np.nonzero((blo < cov_lo) | (bhi > cov_hi))[0]
    return unsafe_a, unsafe_b


def _intervals(a_s, b_s):
    rua = _probe_rub(a_s, b_s)
    rub = _probe_rub(b_s, a_s)
    alo = np.searchsorted(b_s[:, 0], a_s[:, 0] - rua)
    ahi = np.searchsorted(b_s[:, 0], a_s[:, 0] + rua)
    blo = np.searchsorted(a_s[:, 0], b_s[:, 0] - rub)
    bhi = np.searchsorted(a_s[:, 0], b_s[:, 0] + rub)
    return alo, ahi, blo, bhi


def _plan_windows(all_intervals):
    """Choose one per-block (start, width<=W1) table covering, for every
    core, all points that the fixed-W1 baseline covers.  Points left out
    fall to tier 2; the exact per-core unsafe sets are recomputed from the
    final table, so shrinkage never violates the exactness guarantee."""
    base = [(_win_start(i), W1) for i in range(NT)]
    blk = np.arange(N) // 128
    s_need = np.full(NT, N, dtype=np.int64)
    e_need = np.zeros(NT, dtype=np.int64)
    for alo, ahi, _, _ in all_intervals:
        bs = np.array([b[0] for b in base])[blk]
        safe = (alo >= bs) & (ahi <= bs + W1)
        for i in range(NT):
            sel = safe[blk == i]
            if sel.any():
                pts = np.nonzero(blk == i)[0][sel]
                s_need[i] = min(s_need[i], alo[pts].min())
                e_need[i] = max(e_need[i], ahi[pts].max())
    wins = []
    prev_s = 0
    for i in range(NT):
        if s_need[i] >= e_need[i]:
            s, w = base[i]
        else:
            w = min(W1, max(128, -((s_need[i] - e_need[i]) // 64) * 64))
            s = max(0, min(s_need[i] & ~1, N - w))
        s = max(s, prev_s)
        s = min(s, N - w)
        prev_s = s
        wins.append((int(s), int(w)))
    return wins


def _in_maps(array1: np.ndarray, array2: np.ndarray):
    global _WINDOWS
    sorted_pts = []
    all_intervals = []
    for c in range(B):
        a_s = array1[c][np.argsort(array1[c][:, 0], kind="stable")]
        b_s = array2[c][np.argsort(array2[c][:, 0], kind="stable")]
        sorted_pts.append((a_s, b_s))
        all_intervals.append(_intervals(a_s, b_s))

    wins = _plan_windows(all_intervals)
    # widen if any core's tier-2 set exceeds capacity
    for _ in range(4):
        counts = [max(len(u) for u in _unsafe_sets(iv, wins))
                  for iv in all_intervals]
        if max(counts) <= T2:
            break
        wins = [(max(0, min(s - 32, N - min(W1, w + 64))),
                 min(W1, w + 64)) for s, w in wins]
    _WINDOWS = wins

    in_maps = []
    meta = []
    eye = np.eye(128, dtype=np.float32)
    for c in range(B):
        a_s, b_s = sorted_pts[c]
        ua, ub = _unsafe_sets(all_intervals[c], wins)
        assert len(ua) <= T2 and len(ub) <= T2, (len(ua), len(ub))
        wa = _w_side(a_s)
        wb = _w_side(b_s)
        w2a = np.zeros((K, T2), dtype=BF)
        w2b = np.zeros((K, T2), dtype=BF)
        w2a[:, :len(ua)] = wa[:, ua]
        w2b[:, :len(ub)] = wb[:, ub]
        im = {"wa": wa, "mb": _m_side(b_s), "ma": _m_side(a_s),
              "w2a": w2a, "w2b": w2b}
        if not GPS_FINISH:
            im["eye"] = eye
        in_maps.append(im)
        meta.append((ua, ub))
    return in_maps, meta


def kernel(array1: np.ndarray, array2: np.ndarray) -> np.ndarray:
    array1 = np.asarray(array1, dtype=np.float32)
    array2 = np.asarray(array2, dtype=np.float32)
    assert array1.shape == (B, N, 3) and array2.shape == (B, N, 3)

    in_maps, meta = _in_maps(array1, array2)
    nc = _get_nc()
    res = run_bass_kernel_spmd(nc, in_maps, list(range(B))).results

    s1 = 0.0
    s2 = 0.0
    for c in range(B):
        ua, ub = meta[c]
        mina = -res[c]["sa"].astype(np.float64).T.reshape(-1)  # [N] by rank
        if GPS_FINISH:
            minb = -res[c]["sb"].astype(np.float64).reshape(-1)
        else:
            minb = -res[c]["sb"].astype(np.float64).T.reshape(-1)
        t2v = -res[c]["t2"].astype(np.float64)
        mina[ua] = t2v[0, :len(ua)]
        minb[ub] = t2v[1, :len(ub)]
        s1 += np.sqrt(np.maximum(mina, 0.0)).sum()
        s2 += np.sqrt(np.maximum(minb, 0.0)).sum()
    val = 0.5 * (s1 / (B * N) + s2 / (B * N))
    return np.float32(val)
